# revision 31
# baseline (speedup 1.0000x reference)
"""Trainium2 Bass kernel for Convpass-swin hypernet fused adapter.

Reference computation (per batch sample):
  h      = relu(x @ Wm1 + bm1)                    # [B,H,W,64]
  prompt = mean_hw(h) @ Wm2 + bm2                 # [B,64]  (mean commutes with matmul)
  wflat  = (emb + prompt) @ Wh + bh               # [B,96*96*9]
  xd     = quickgelu(x @ Wd + bd)                 # [B,H,W,96]
  y      = quickgelu(conv3x3(xd, wflat))          # per-sample dynamic grouped conv
  out    = y @ Wu + bu                            # [B,H,W,384]

Sharding: data-parallel over batch B=64 across 8 cores (8 samples/core),
weights replicated.

Key layout/precision choices (v2):
  - x is transposed to channel-major [128, 3, BL*P] on the host and streamed
    as bf16 (halves DMA, removes all on-device transposes).
  - The hypernet is split as wflat = wbase + prompt @ Wh where
    wbase = (emb + bm2) @ Wh + bh is computed exactly on the host.  Only the
    small dynamic part runs on-device, which lets Wh stream in fp8 (e4m3,
    x32 scale) at half the bf16 DMA cost without blowing the error budget.
  - Wh columns are permuted to (tap, o, i) order so each 3x3 tap's weights
    complete early in the stream; the conv for a first wave of samples
    "chases" the stream tap by tap while later samples replay from SBUF.
    The remaining samples' adapter-down matmuls are interleaved into the
    stream loop to keep the PE busy between chunks.
  - The unadapter output is produced channel-major ([3,128] chunks of C) so
    Wu can be the stationary operand; bu is applied via the PSUM->SBUF copy
    bias.  The host transposes the bf16 result back.
"""
import numpy as np
import ml_dtypes

import concourse.bass as bass
import concourse.tile as tile
import concourse.mybir as mybir
from concourse import bacc
from concourse.bass_utils import run_bass_kernel_spmd

F32 = mybir.dt.float32
BF = mybir.dt.bfloat16
F8 = mybir.dt.float8e4
AF = mybir.ActivationFunctionType
AX = mybir.AxisListType
OP = mybir.AluOpType

# problem constants
B, H, W, C = 64, 28, 28, 384
DIM, E, KK = 96, 64, 3
NCORES = 8
BL = B // NCORES          # samples per core
P = H * W                 # 784 positions per sample
NPOS = BL * P             # 6272 positions per core
NTO = DIM * 9             # 864 (tap, o) pairs
WH_COLS = DIM * NTO       # 82944

WH_FP8 = True             # stream hypernet matrix in fp8 (needs wbase split)
WH_SCALE = 32.0           # Wh pre-scale (keeps fp8 values in normal range)
PV_SCALE = 8.0            # prompt pre-scale
W_SCALE = (WH_SCALE * PV_SCALE) if WH_FP8 else 1.0   # scale carried by w_all

PAIRS_PER_CHUNK = 96      # hypernet (t,o) pairs per Wh DMA chunk (= one tap)
NCHUNK = NTO // PAIRS_PER_CHUNK      # 9
CHUNK_COLS = PAIRS_PER_CHUNK * DIM   # 9216
GRP = 48                  # (t,o) pairs per PSUM group (bank limit)
# conv half-sample tiles [96, 392] that chase the Wh stream (PSUM banks:
# len(CHASE) + pw bufs must be <= 8)
CHASE = [(0, 0), (0, 1), (1, 0), (1, 1), (2, 0)]


def build_nc():
    nc = bacc.Bacc("TRN2", target_bir_lowering=False, debug=False)

    xT_d = nc.dram_tensor("xT", [128, C // 128, NPOS], BF, kind="ExternalInput").ap()
    wm1_d = nc.dram_tensor("wm1", [128, C // 128, E], BF, kind="ExternalInput").ap()
    wd_d = nc.dram_tensor("wd", [128, C // 128, DIM], BF, kind="ExternalInput").ap()
    wm2_d = nc.dram_tensor("wm2", [E, E], BF, kind="ExternalInput").ap()
    wu_d = nc.dram_tensor("wu", [DIM, C], BF, kind="ExternalInput").ap()
    wh_d = nc.dram_tensor("wh", [E, WH_COLS], F8 if WH_FP8 else BF,
                          kind="ExternalInput").ap()
    wbase_d = nc.dram_tensor("wbase", [DIM, NTO, 1], BF, kind="ExternalInput").ap()
    # fused small biases: col 0 = bm1 (rows 0:64), col 1 = bd (rows 0:96),
    # cols 2:5 = bu in three 128-row chunks
    bias_d = nc.dram_tensor("bias", [128, 5], F32, kind="ExternalInput").ap()
    out_d = nc.dram_tensor("out", [128, C // 128, NPOS], BF, kind="ExternalOutput").ap()

    with tile.TileContext(nc) as tc:
        with (
            tc.tile_pool(name="const", bufs=1) as cp,
            tc.tile_pool(name="persist", bufs=1) as pp,
            tc.tile_pool(name="wh", bufs=9) as wh_p,
            tc.tile_pool(name="outp", bufs=3) as out_p,
        ):
            # ---- persistent state ----
            xT_sb = pp.tile([128, C // 128, NPOS], BF)
            xd_pad = pp.tile([DIM, BL, H + 2, W + 2], BF)
            w_all = pp.tile([DIM, NTO, BL], BF)      # [i, (t,o), b] * W_SCALE
            y_sb = pp.tile([DIM, BL, P], BF)
            hsum = pp.tile([E, BL, 2], F32)
            hbar = pp.tile([E, BL], F32)
            hbar_r = pp.tile([E, BL], BF)
            pvec = pp.tile([E, BL], F8 if WH_FP8 else BF)
            h_scr = pp.tile([E, 392], F32)

            # ---- DMA order: h/xd weights, x per sample, rest, Wh chunks ----
            wm1_sb = cp.tile([128, C // 128, E], BF)
            bias_sb = cp.tile([128, 5], F32)
            wd_sb = cp.tile([128, C // 128, DIM], BF)
            wm2_sb = cp.tile([E, E], BF)
            wu_sb = cp.tile([DIM, C], BF)
            wbase_sb = cp.tile([DIM, NTO, 1], BF)

            nc.sync.dma_start(xT_sb[:, :, 0:392], xT_d[:, :, 0:392])
            nc.sync.dma_start(wm1_sb[:], wm1_d)
            nc.sync.dma_start(bias_sb[:], bias_d)
            nc.sync.dma_start(xT_sb[:, :, 392:P], xT_d[:, :, 392:P])
            nc.sync.dma_start(wd_sb[:], wd_d)
            for b in range(1, BL):
                nc.sync.dma_start(xT_sb[:, :, b * P:(b + 1) * P],
                                    xT_d[:, :, b * P:(b + 1) * P])
            nc.sync.dma_start(wm2_sb[:], wm2_d)
            nc.sync.dma_start(wbase_sb[:], wbase_d)
            nc.sync.dma_start(wu_sb[:], wu_d)

            bm1 = bias_sb[0:E, 0:1]
            bd = bias_sb[0:DIM, 1:2]

            # zero the conv halo borders of xd_pad (interior is overwritten)
            nc.vector.memset(xd_pad[:, :, 0:30:29, :], 0.0)
            nc.vector.memset(xd_pad[:, :, 1:29, 0:30:29], 0.0)
            zeros = cp.tile([E, 1], F32)
            nc.vector.memset(zeros[:], 0.0)
            h_scr2 = pp.tile([E, 392], F32)

            # ---- per sample: meta-net h sums + xd = quickgelu(x@Wd+bd).
            # ppm (the prompt matmul) is issued before the last xd half so
            # pvec is ready the moment the h sums complete. ----
            hp_ctx = tc.tile_pool(name="ps_h", bufs=3, space="PSUM")
            ps_h = hp_ctx.__enter__()
            xp_ctx = tc.tile_pool(name="ps_x", bufs=3, space="PSUM")
            ps_x = xp_ctx.__enter__()

            def make_h(b, h2):
                lo = b * P + h2 * 392
                ph = ps_h.tile([E, 392], F32, name="ph", tag="ph")
                for c in range(C // 128):
                    nc.tensor.matmul(ph[:], wm1_sb[:, c, :], xT_sb[:, c, lo:lo + 392],
                                     start=(c == 0), stop=(c == 2))
                # relu+bias+spatial-sum on DVE (ACT is busy with gelus,
                # and pvec is on the critical path)
                nc.vector.scalar_tensor_tensor(
                    h_scr2[:], ph[:], bm1, zeros[:].to_broadcast((E, 392)),
                    op0=OP.add, op1=OP.max,
                    accum_out=hsum[:, b, h2:h2 + 1])

            def make_xd(b, h2):
                lo = b * P + h2 * 392
                px = ps_x.tile([DIM, 392], F32, name="px", tag="px")
                for c in range(C // 128):
                    nc.tensor.matmul(px[:], wd_sb[:, c, :], xT_sb[:, c, lo:lo + 392],
                                     start=(c == 0), stop=(c == 2))
                nc.scalar.activation(
                    xd_pad[:, b, 1 + h2 * 14: 15 + h2 * 14, 1:29],
                    px[:].rearrange("p (r c) -> p r c", r=14),
                    AF.Gelu_apprx_sigmoid, bias=bd)

            for b in range(BL):
                if b < BL - 1:
                    for h2 in range(2):
                        make_h(b, h2)
                        make_xd(b, h2)
                else:
                    make_h(b, 0)
                    make_h(b, 1)

            # ---- prompt -> pvec (scaled; no emb/bias: wbase covers them) ----
            nc.vector.reduce_sum(hbar[:], hsum[:], axis=AX.X)
            nc.scalar.activation(hbar_r[:], hbar[:], AF.Copy, scale=1.0 / P)
            make_xd(BL - 1, 0)
            ppm = ps_h.tile([E, BL], F32, name="ppm", tag="ppm", bufs=1)
            nc.tensor.matmul(ppm[:], wm2_sb[:], hbar_r[:], start=True, stop=True)
            nc.scalar.activation(pvec[:], ppm[:], AF.Copy, scale=PV_SCALE)
            make_xd(BL - 1, 1)
            xp_ctx.__exit__(None, None, None)
            hp_ctx.__exit__(None, None, None)

            # ---- Wh stream: hypernet + wbase add + wave-A conv chase ----
            pca_ctx = tc.tile_pool(name="ps_ca", bufs=1, space="PSUM")
            ps_ca = pca_ctx.__enter__()
            pca = [ps_ca.tile([DIM, 392], F32, name=f"pca{k}") for k in range(len(CHASE))]
            pw_ctx = tc.tile_pool(name="ps_w", bufs=3, space="PSUM")
            ps_w = pw_ctx.__enter__()

            def conv_half(pc, s, h2, t):
                dy, dx = t // 3, t % 3
                nc.tensor.matmul(
                    pc,
                    w_all[:, t * DIM:(t + 1) * DIM, s],
                    xd_pad[:, s, h2 * 14 + dy: h2 * 14 + dy + 14, dx:dx + 28],
                    start=(t == 0), stop=(t == 8))

            def chase(t):
                for k, (s, h2) in enumerate(CHASE):
                    conv_half(pca[k], s, h2, t)

            for jc in range(NCHUNK):
              # logical-phase barrier: force the scheduler to emit each
              # iteration's PE work in source order so a chase waiting on its
              # wbase-add never head-of-line-blocks ready hypernet matmuls
              with tc.tile_wait_until(ms=jc + 1):
                whc = wh_p.tile([E, CHUNK_COLS], F8 if WH_FP8 else BF, tag="whc")
                nc.sync.dma_start(whc[:], wh_d[:, jc * CHUNK_COLS:(jc + 1) * CHUNK_COLS])
                for half in range(PAIRS_PER_CHUNK // GRP):
                    pw = ps_w.tile([DIM, GRP * BL], F32, name="pw", tag="pw")
                    for g in range(GRP):
                        gg = half * GRP + g
                        nc.tensor.matmul(pw[:, g * BL:(g + 1) * BL],
                                         whc[:, gg * DIM:(gg + 1) * DIM], pvec[:],
                                         start=True, stop=True)
                    g0 = jc * PAIRS_PER_CHUNK + half * GRP
                    nc.vector.scalar_tensor_tensor(
                        w_all[:, g0:g0 + GRP, :],
                        pw[:].rearrange("p (g b) -> p g b", g=GRP),
                        1.0,
                        wbase_sb[:, g0:g0 + GRP, :].to_broadcast((DIM, GRP, BL)),
                        op0=OP.mult, op1=OP.add)
                # conv chase, lagging two taps so it never waits on the
                # wbase-add DVE chain of the chunk just streamed
                if jc > 1:
                    chase(jc - 2)
            with tc.tile_wait_until(ms=NCHUNK + 1):
                chase(NCHUNK - 2)
                chase(NCHUNK - 1)
            pw_ctx.__exit__(None, None, None)

            # ---- finish: gelu + unadapter + out at half-sample granularity.
            # una halves are staggered behind the conv so they fill PE stalls
            # while ACT runs the gelus. ----
            po_ctx = tc.tile_pool(name="ps_o", bufs=3, space="PSUM")
            ps_o = po_ctx.__enter__()
            neng = 0
            ob_tiles = {}

            def gelu_half(pc, s, h2):
                nc.scalar.activation(y_sb[:, s, h2 * 392:(h2 + 1) * 392],
                                     pc, AF.Gelu_apprx_sigmoid,
                                     scale=1.0 / W_SCALE)

            def una_half(s, h2):
                nonlocal neng
                if s not in ob_tiles:
                    ob_tiles[s] = out_p.tile([128, C // 128, P], BF, tag="ob",
                                             name=f"ob{s}")
                ob = ob_tiles[s]
                for q in range(C // 128):
                    po = ps_o.tile([128, 392], F32, name="po", tag="po")
                    nc.tensor.matmul(po[:], wu_sb[:, q * 128:(q + 1) * 128],
                                     y_sb[:, s, h2 * 392:(h2 + 1) * 392],
                                     start=True, stop=True)
                    dst = ob[:, q, h2 * 392:(h2 + 1) * 392]
                    if neng % 2 == 0:
                        nc.scalar.activation(dst, po[:], AF.Identity,
                                             bias=bias_sb[:, 2 + q:3 + q])
                    else:
                        nc.vector.scalar_tensor_tensor(
                            dst, po[:], 1.0,
                            bias_sb[:, 2 + q:3 + q].to_broadcast((128, 392)),
                            op0=OP.mult, op1=OP.add)
                    neng += 1
                    if h2 == 1:
                        nc.sync.dma_start(out_d[:, q, s * P:(s + 1) * P],
                                          ob[:, q, :])

            for k, (s, h2) in enumerate(CHASE):
                gelu_half(pca[k], s, h2)
            pend = list(CHASE)
            wave_b = [sh for s in range(BL) for sh in ((s, 0), (s, 1))
                      if sh not in CHASE]
            for k, (s, h2) in enumerate(wave_b):
                pc = pca[k % len(pca)]
                for t in range(9):
                    conv_half(pc, s, h2, t)
                gelu_half(pc, s, h2)
                for _ in range(2 if len(pend) > 2 else 1):
                    if pend:
                        una_half(*pend.pop(0))
                pend.append((s, h2))
            for s, h2 in pend:
                una_half(s, h2)
            po_ctx.__exit__(None, None, None)
            pca_ctx.__exit__(None, None, None)

    nc.compile()
    return nc


_NC_CACHE = None


def _get_nc():
    global _NC_CACHE
    if _NC_CACHE is None:
        _NC_CACHE = build_nc()
    return _NC_CACHE


def _prep_inputs(x, Wd, bd, Wm1, bm1, Wm2, bm2, Wh, bh, emb, Wu, bu):
    """Host-side prep: permute weights, precompute wbase, shard x."""
    f32 = np.float32
    bf16 = ml_dtypes.bfloat16

    # Wh columns: original order (o, i, t) -> (t, o, i)
    whp = np.asarray(Wh, f32).reshape(E, DIM, DIM, 9)          # (e, o, i, t)
    whp = whp.transpose(0, 3, 1, 2).reshape(E, WH_COLS)        # (e, (t, o, i))
    # wbase = (emb + bm2) @ Wh + bh, in [i, (t, o)] layout, carrying W_SCALE
    wb = (np.asarray(emb, f32) + np.asarray(bm2, f32)) @ np.asarray(Wh, f32) \
        + np.asarray(bh, f32)                                  # [(o, i, t)]
    wb = wb.reshape(DIM, DIM, 9).transpose(1, 2, 0)            # (i, t, o)
    wb = np.ascontiguousarray(wb.reshape(DIM, NTO, 1) * W_SCALE)

    if WH_FP8:
        wh_up = (whp * WH_SCALE).astype(ml_dtypes.float8_e4m3)
    else:
        wh_up = whp.astype(bf16)

    bias = np.zeros((128, 5), f32)
    bias[:E, 0] = np.asarray(bm1, f32)
    bias[:DIM, 1] = np.asarray(bd, f32)
    bias[:, 2:5] = np.asarray(bu, f32).reshape(C // 128, 128).T

    def cmajor(a):  # [C, k] -> [128, 3, k]
        return np.ascontiguousarray(
            np.asarray(a, f32).reshape(C // 128, 128, -1).transpose(1, 0, 2)
        ).astype(bf16)

    shared = {
        "wm1": cmajor(Wm1),
        "wd": cmajor(Wd),
        "wm2": np.asarray(Wm2, f32).astype(bf16),
        "wu": np.asarray(Wu, f32).astype(bf16),
        "wh": wh_up,
        "wbase": wb.astype(bf16),
        "bias": bias,
    }
    xs = np.asarray(x, f32).reshape(B, P, C)
    in_maps = []
    for k in range(NCORES):
        m = dict(shared)
        xc = xs[k * BL:(k + 1) * BL].reshape(NPOS, C)          # [n, c]
        xc = xc.T.reshape(C // 128, 128, NPOS).transpose(1, 0, 2)
        m["xT"] = np.ascontiguousarray(xc).astype(bf16)
        in_maps.append(m)
    return in_maps


def _run(inputs, **spmd_kwargs):
    nc = _get_nc()
    in_maps = _prep_inputs(**inputs)
    res = run_bass_kernel_spmd(nc, in_maps, core_ids=list(range(NCORES)), **spmd_kwargs)
    outs = []
    for r in res.results:
        o = np.asarray(r["out"], dtype=np.float32)             # [128, 3, NPOS]
        o = o.transpose(1, 0, 2).reshape(C, NPOS).T            # [NPOS, C]
        outs.append(o)
    out = np.concatenate(outs, 0)
    return out.reshape(B, H, W, C), res


def kernel(**inputs) -> np.ndarray:
    out, _ = _run(inputs)
    return out


# revision 41
# speedup vs baseline: 1.0221x; 1.0221x over previous
"""Trainium2 Bass kernel for Convpass-swin hypernet fused adapter.

Reference computation (per batch sample):
  h      = relu(x @ Wm1 + bm1)                    # [B,H,W,64]
  prompt = mean_hw(h) @ Wm2 + bm2                 # [B,64]  (mean commutes with matmul)
  wflat  = (emb + prompt) @ Wh + bh               # [B,96*96*9]
  xd     = quickgelu(x @ Wd + bd)                 # [B,H,W,96]
  y      = quickgelu(conv3x3(xd, wflat))          # per-sample dynamic grouped conv
  out    = y @ Wu + bu                            # [B,H,W,384]

Sharding: data-parallel over batch B=64 across 8 cores (8 samples/core),
weights replicated.

Key layout/precision choices (v2):
  - x is transposed to channel-major [128, 3, BL*P] on the host and streamed
    as bf16 (halves DMA, removes all on-device transposes).
  - The hypernet is split as wflat = wbase + prompt @ Wh where
    wbase = (emb + bm2) @ Wh + bh is computed exactly on the host.  Only the
    small dynamic part runs on-device, which lets Wh stream in fp8 (e4m3,
    x32 scale) at half the bf16 DMA cost without blowing the error budget.
  - Wh columns are permuted to (tap, o, i) order so each 3x3 tap's weights
    complete early in the stream; the conv for the first 2.5 samples
    "chases" the stream tap by tap (PSUM-bank limited) while the remaining
    halves replay from SBUF afterwards.  Scheduler phase barriers
    (tile_wait_until) keep a chase that waits on the wbase-add DVE chain
    from head-of-line-blocking ready hypernet matmuls.
  - The unadapter output is produced channel-major ([3,128] chunks of C) so
    Wu can be the stationary operand; bu is applied via the PSUM->SBUF copy
    bias; una halves are staggered behind the wave-B convs as PE fillers.
    The host transposes the bf16 result back.
"""
import numpy as np
import ml_dtypes

import concourse.bass as bass
import concourse.tile as tile
import concourse.mybir as mybir
from concourse import bacc
from concourse.bass_utils import run_bass_kernel_spmd

F32 = mybir.dt.float32
BF = mybir.dt.bfloat16
F8 = mybir.dt.float8e4
AF = mybir.ActivationFunctionType
AX = mybir.AxisListType
OP = mybir.AluOpType

# problem constants
B, H, W, C = 64, 28, 28, 384
DIM, E, KK = 96, 64, 3
NCORES = 8
BL = B // NCORES          # samples per core
P = H * W                 # 784 positions per sample
NPOS = BL * P             # 6272 positions per core
NTO = DIM * 9             # 864 (tap, o) pairs
WH_COLS = DIM * NTO       # 82944

WH_FP8 = True             # stream hypernet matrix in fp8 (needs wbase split)
WH_SCALE = 32.0           # Wh pre-scale (keeps fp8 values in normal range)
PV_SCALE = 8.0            # prompt pre-scale
W_SCALE = (WH_SCALE * PV_SCALE) if WH_FP8 else 1.0   # scale carried by w_all

PAIRS_PER_CHUNK = 96      # hypernet (t,o) pairs per Wh DMA chunk (= one tap)
NCHUNK = NTO // PAIRS_PER_CHUNK      # 9
CHUNK_COLS = PAIRS_PER_CHUNK * DIM   # 9216
GRP = 48                  # (t,o) pairs per PSUM group (bank limit)
# conv half-sample tiles [96, 392] that chase the Wh stream (PSUM banks:
# len(CHASE) + pw bufs must be <= 8)
CHASE = [(0, 0), (0, 1), (1, 0), (1, 1), (2, 0)]


def build_nc():
    nc = bacc.Bacc("TRN2", target_bir_lowering=False, debug=False)

    xT_d = nc.dram_tensor("xT", [128, C // 128, NPOS], BF, kind="ExternalInput").ap()
    wm1_d = nc.dram_tensor("wm1", [128, C // 128, E], BF, kind="ExternalInput").ap()
    wd_d = nc.dram_tensor("wd", [128, C // 128, DIM], BF, kind="ExternalInput").ap()
    wm2_d = nc.dram_tensor("wm2", [E, E], BF, kind="ExternalInput").ap()
    wu_d = nc.dram_tensor("wu", [DIM, C], BF, kind="ExternalInput").ap()
    wh_d = nc.dram_tensor("wh", [E, WH_COLS], F8 if WH_FP8 else BF,
                          kind="ExternalInput").ap()
    wbase_d = nc.dram_tensor("wbase", [DIM, NTO, 1], BF, kind="ExternalInput").ap()
    # fused small biases: col 0 = bm1 (rows 0:64), col 1 = bd (rows 0:96),
    # cols 2:5 = bu in three 128-row chunks
    bias_d = nc.dram_tensor("bias", [128, 5], F32, kind="ExternalInput").ap()
    out_d = nc.dram_tensor("out", [128, C // 128, NPOS], BF, kind="ExternalOutput").ap()

    with tile.TileContext(nc) as tc:
        with (
            tc.tile_pool(name="const", bufs=1) as cp,
            tc.tile_pool(name="persist", bufs=1) as pp,
            tc.tile_pool(name="wh", bufs=9) as wh_p,
            tc.tile_pool(name="outp", bufs=3) as out_p,
        ):
            # ---- persistent state ----
            xT_sb = pp.tile([128, C // 128, NPOS], BF)
            xd_pad = pp.tile([DIM, BL, H + 2, W + 2], BF)
            w_all = pp.tile([DIM, NTO, BL], BF)      # [i, (t,o), b] * W_SCALE
            y_sb = pp.tile([DIM, BL, P], BF)
            hsum = pp.tile([E, BL, 2], F32)
            hbar = pp.tile([E, BL], F32)
            hbar_r = pp.tile([E, BL], BF)
            pvec = pp.tile([E, BL], F8 if WH_FP8 else BF)
            h_scr = pp.tile([E, 392], F32)

            # ---- DMA order: h/xd weights, x per sample, rest, Wh chunks ----
            wm1_sb = cp.tile([128, C // 128, E], BF)
            bias_sb = cp.tile([128, 5], F32)
            wd_sb = cp.tile([128, C // 128, DIM], BF)
            wm2_sb = cp.tile([E, E], BF)
            wu_sb = cp.tile([DIM, C], BF)
            wbase_sb = cp.tile([DIM, NTO, 1], BF)

            nc.sync.dma_start(xT_sb[:, :, 0:196], xT_d[:, :, 0:196])
            nc.sync.dma_start(wm1_sb[:], wm1_d)
            nc.sync.dma_start(xT_sb[:, :, 196:392], xT_d[:, :, 196:392])
            nc.sync.dma_start(bias_sb[:], bias_d)
            nc.sync.dma_start(xT_sb[:, :, 392:P], xT_d[:, :, 392:P])
            nc.sync.dma_start(wd_sb[:], wd_d)
            for b in range(1, BL):
                nc.sync.dma_start(xT_sb[:, :, b * P:(b + 1) * P],
                                    xT_d[:, :, b * P:(b + 1) * P])
            nc.sync.dma_start(wm2_sb[:], wm2_d)
            nc.sync.dma_start(wbase_sb[:], wbase_d)
            nc.sync.dma_start(wu_sb[:], wu_d)

            bm1 = bias_sb[0:E, 0:1]
            bd = bias_sb[0:DIM, 1:2]

            # zero the conv halo borders of xd_pad (interior is overwritten)
            nc.vector.memset(xd_pad[:, :, 0:30:29, :], 0.0)
            nc.vector.memset(xd_pad[:, :, 1:29, 0:30:29], 0.0)
            zeros = cp.tile([E, 1], F32)
            nc.vector.memset(zeros[:], 0.0)
            h_scr2 = pp.tile([E, 392], F32)

            # ---- per sample: meta-net h sums + xd = quickgelu(x@Wd+bd).
            # ppm (the prompt matmul) is issued before the last xd half so
            # pvec is ready the moment the h sums complete. ----
            hp_ctx = tc.tile_pool(name="ps_h", bufs=3, space="PSUM")
            ps_h = hp_ctx.__enter__()
            xp_ctx = tc.tile_pool(name="ps_x", bufs=3, space="PSUM")
            ps_x = xp_ctx.__enter__()

            def make_h(b, h2):
                lo = b * P + h2 * 392
                ph = ps_h.tile([E, 392], F32, name="ph", tag="ph")
                for c in range(C // 128):
                    nc.tensor.matmul(ph[:], wm1_sb[:, c, :], xT_sb[:, c, lo:lo + 392],
                                     start=(c == 0), stop=(c == 2))
                # relu+bias+spatial-sum on DVE (ACT is busy with gelus,
                # and pvec is on the critical path)
                nc.vector.scalar_tensor_tensor(
                    h_scr2[:], ph[:], bm1, zeros[:].to_broadcast((E, 392)),
                    op0=OP.add, op1=OP.max,
                    accum_out=hsum[:, b, h2:h2 + 1])

            def make_xd(b, h2):
                lo = b * P + h2 * 392
                px = ps_x.tile([DIM, 392], F32, name="px", tag="px")
                for c in range(C // 128):
                    nc.tensor.matmul(px[:], wd_sb[:, c, :], xT_sb[:, c, lo:lo + 392],
                                     start=(c == 0), stop=(c == 2))
                nc.scalar.activation(
                    xd_pad[:, b, 1 + h2 * 14: 15 + h2 * 14, 1:29],
                    px[:].rearrange("p (r c) -> p r c", r=14),
                    AF.Gelu_apprx_sigmoid, bias=bd)

            # first h half runs as two 196-column pieces so the PE starts as
            # soon as the first quarter of sample 0 lands
            with tc.high_priority():
                ph0 = ps_h.tile([E, 392], F32, name="ph", tag="ph")
                for piece in range(2):
                    for c in range(C // 128):
                        nc.tensor.matmul(
                            ph0[:, piece * 196:(piece + 1) * 196],
                            wm1_sb[:, c, :],
                            xT_sb[:, c, piece * 196:(piece + 1) * 196],
                            start=(c == 0), stop=(c == 2))
                nc.vector.scalar_tensor_tensor(
                    h_scr2[:], ph0[:], bm1, zeros[:].to_broadcast((E, 392)),
                    op0=OP.add, op1=OP.max, accum_out=hsum[:, 0, 0:1])
            make_xd(0, 0)
            for b in range(BL):
                if b < BL - 1:
                    for h2 in range(2):
                        if b == 0 and h2 == 0:
                            continue
                        with tc.high_priority():
                            make_h(b, h2)
                        make_xd(b, h2)
                else:
                    with tc.high_priority():
                        make_h(b, 0)
                        make_h(b, 1)

            # ---- prompt -> pvec (scaled; no emb/bias: wbase covers them) ----
            with tc.high_priority():
                nc.vector.reduce_sum(hbar[:], hsum[:], axis=AX.X)
                nc.scalar.activation(hbar_r[:], hbar[:], AF.Copy, scale=1.0 / P)
            make_xd(BL - 1, 0)
            with tc.high_priority():
                ppm = ps_h.tile([E, BL], F32, name="ppm", tag="ppm", bufs=1)
                nc.tensor.matmul(ppm[:], wm2_sb[:], hbar_r[:], start=True, stop=True)
                nc.scalar.activation(pvec[:], ppm[:], AF.Copy, scale=PV_SCALE)
            make_xd(BL - 1, 1)
            xp_ctx.__exit__(None, None, None)
            hp_ctx.__exit__(None, None, None)

            # ---- Wh stream: hypernet + wbase add + wave-A conv chase ----
            pca_ctx = tc.tile_pool(name="ps_ca", bufs=1, space="PSUM")
            ps_ca = pca_ctx.__enter__()
            pca = [ps_ca.tile([DIM, 392], F32, name=f"pca{k}") for k in range(len(CHASE))]
            pw_ctx = tc.tile_pool(name="ps_w", bufs=3, space="PSUM")
            ps_w = pw_ctx.__enter__()

            def conv_half(pc, s, h2, t):
                dy, dx = t // 3, t % 3
                nc.tensor.matmul(
                    pc,
                    w_all[:, t * DIM:(t + 1) * DIM, s],
                    xd_pad[:, s, h2 * 14 + dy: h2 * 14 + dy + 14, dx:dx + 28],
                    start=(t == 0), stop=(t == 8))

            def chase(t):
                for k, (s, h2) in enumerate(CHASE):
                    conv_half(pca[k], s, h2, t)

            for jc in range(NCHUNK):
              # logical-phase barrier: force the scheduler to emit each
              # iteration's PE work in source order so a chase waiting on its
              # wbase-add never head-of-line-blocks ready hypernet matmuls
              with tc.tile_wait_until(ms=jc + 1):
                whc = wh_p.tile([E, CHUNK_COLS], F8 if WH_FP8 else BF, tag="whc")
                nc.sync.dma_start(whc[:], wh_d[:, jc * CHUNK_COLS:(jc + 1) * CHUNK_COLS])
                for half in range(PAIRS_PER_CHUNK // GRP):
                    pw = ps_w.tile([DIM, GRP * BL], F32, name="pw", tag="pw")
                    for g in range(GRP):
                        gg = half * GRP + g
                        nc.tensor.matmul(pw[:, g * BL:(g + 1) * BL],
                                         whc[:, gg * DIM:(gg + 1) * DIM], pvec[:],
                                         start=True, stop=True)
                    g0 = jc * PAIRS_PER_CHUNK + half * GRP
                    nc.vector.scalar_tensor_tensor(
                        w_all[:, g0:g0 + GRP, :],
                        pw[:].rearrange("p (g b) -> p g b", g=GRP),
                        1.0,
                        wbase_sb[:, g0:g0 + GRP, :].to_broadcast((DIM, GRP, BL)),
                        op0=OP.mult, op1=OP.add)
                # conv chase, lagging two taps so it never waits on the
                # wbase-add DVE chain of the chunk just streamed
                if jc > 1:
                    chase(jc - 2)
            with tc.tile_wait_until(ms=NCHUNK + 1):
                chase(NCHUNK - 2)
                chase(NCHUNK - 1)
            pw_ctx.__exit__(None, None, None)

            # ---- finish: gelu + unadapter + out at half-sample granularity.
            # una halves are staggered behind the conv so they fill PE stalls
            # while ACT runs the gelus. ----
            po_ctx = tc.tile_pool(name="ps_o", bufs=3, space="PSUM")
            ps_o = po_ctx.__enter__()
            neng = 0
            ob_tiles = {}

            def gelu_half(pc, s, h2):
                nc.scalar.activation(y_sb[:, s, h2 * 392:(h2 + 1) * 392],
                                     pc, AF.Gelu_apprx_sigmoid,
                                     scale=1.0 / W_SCALE)

            def una_half(s, h2):
                nonlocal neng
                if s not in ob_tiles:
                    ob_tiles[s] = out_p.tile([128, C // 128, P], BF, tag="ob",
                                             name=f"ob{s}")
                ob = ob_tiles[s]
                for q in range(C // 128):
                    po = ps_o.tile([128, 392], F32, name="po", tag="po")
                    nc.tensor.matmul(po[:], wu_sb[:, q * 128:(q + 1) * 128],
                                     y_sb[:, s, h2 * 392:(h2 + 1) * 392],
                                     start=True, stop=True)
                    dst = ob[:, q, h2 * 392:(h2 + 1) * 392]
                    if neng % 2 == 0:
                        nc.scalar.activation(dst, po[:], AF.Identity,
                                             bias=bias_sb[:, 2 + q:3 + q])
                    else:
                        nc.vector.scalar_tensor_tensor(
                            dst, po[:], 1.0,
                            bias_sb[:, 2 + q:3 + q].to_broadcast((128, 392)),
                            op0=OP.mult, op1=OP.add)
                    neng += 1
                    if h2 == 1:
                        nc.sync.dma_start(out_d[:, q, s * P:(s + 1) * P],
                                          ob[:, q, :])

            for k, (s, h2) in enumerate(CHASE):
                gelu_half(pca[k], s, h2)
            pend = list(CHASE)
            wave_b = [sh for s in range(BL) for sh in ((s, 0), (s, 1))
                      if sh not in CHASE]
            for k, (s, h2) in enumerate(wave_b):
                pc = pca[k % len(pca)]
                for t in range(9):
                    conv_half(pc, s, h2, t)
                gelu_half(pc, s, h2)
                for _ in range(2 if len(pend) > 1 else 1):
                    if pend:
                        una_half(*pend.pop(0))
                pend.append((s, h2))
            for s, h2 in pend:
                una_half(s, h2)
            po_ctx.__exit__(None, None, None)
            pca_ctx.__exit__(None, None, None)

    nc.compile()
    return nc


_NC_CACHE = None


def _get_nc():
    global _NC_CACHE
    if _NC_CACHE is None:
        _NC_CACHE = build_nc()
    return _NC_CACHE


def _prep_inputs(x, Wd, bd, Wm1, bm1, Wm2, bm2, Wh, bh, emb, Wu, bu):
    """Host-side prep: permute weights, precompute wbase, shard x."""
    f32 = np.float32
    bf16 = ml_dtypes.bfloat16

    # Wh columns: original order (o, i, t) -> (t, o, i)
    whp = np.asarray(Wh, f32).reshape(E, DIM, DIM, 9)          # (e, o, i, t)
    whp = whp.transpose(0, 3, 1, 2).reshape(E, WH_COLS)        # (e, (t, o, i))
    # wbase = (emb + bm2) @ Wh + bh, in [i, (t, o)] layout, carrying W_SCALE
    wb = (np.asarray(emb, f32) + np.asarray(bm2, f32)) @ np.asarray(Wh, f32) \
        + np.asarray(bh, f32)                                  # [(o, i, t)]
    wb = wb.reshape(DIM, DIM, 9).transpose(1, 2, 0)            # (i, t, o)
    wb = np.ascontiguousarray(wb.reshape(DIM, NTO, 1) * W_SCALE)

    if WH_FP8:
        wh_up = (whp * WH_SCALE).astype(ml_dtypes.float8_e4m3)
    else:
        wh_up = whp.astype(bf16)

    bias = np.zeros((128, 5), f32)
    bias[:E, 0] = np.asarray(bm1, f32)
    bias[:DIM, 1] = np.asarray(bd, f32)
    bias[:, 2:5] = np.asarray(bu, f32).reshape(C // 128, 128).T

    def cmajor(a):  # [C, k] -> [128, 3, k]
        return np.ascontiguousarray(
            np.asarray(a, f32).reshape(C // 128, 128, -1).transpose(1, 0, 2)
        ).astype(bf16)

    shared = {
        "wm1": cmajor(Wm1),
        "wd": cmajor(Wd),
        "wm2": np.asarray(Wm2, f32).astype(bf16),
        "wu": np.asarray(Wu, f32).astype(bf16),
        "wh": wh_up,
        "wbase": wb.astype(bf16),
        "bias": bias,
    }
    xs = np.asarray(x, f32).reshape(B, P, C)
    in_maps = []
    for k in range(NCORES):
        m = dict(shared)
        xc = xs[k * BL:(k + 1) * BL].reshape(NPOS, C)          # [n, c]
        xc = xc.T.reshape(C // 128, 128, NPOS).transpose(1, 0, 2)
        m["xT"] = np.ascontiguousarray(xc).astype(bf16)
        in_maps.append(m)
    return in_maps


def _run(inputs, **spmd_kwargs):
    nc = _get_nc()
    in_maps = _prep_inputs(**inputs)
    res = run_bass_kernel_spmd(nc, in_maps, core_ids=list(range(NCORES)), **spmd_kwargs)
    outs = []
    for r in res.results:
        o = np.asarray(r["out"], dtype=np.float32)             # [128, 3, NPOS]
        o = o.transpose(1, 0, 2).reshape(C, NPOS).T            # [NPOS, C]
        outs.append(o)
    out = np.concatenate(outs, 0)
    return out.reshape(B, H, W, C), res


def kernel(**inputs) -> np.ndarray:
    out, _ = _run(inputs)
    return out


# revision 50
# speedup vs baseline: 1.0282x; 1.0060x over previous
"""Trainium2 Bass kernel for Convpass-swin hypernet fused adapter.

Reference computation (per batch sample):
  h      = relu(x @ Wm1 + bm1)                    # [B,H,W,64]
  prompt = mean_hw(h) @ Wm2 + bm2                 # [B,64]  (mean commutes with matmul)
  wflat  = (emb + prompt) @ Wh + bh               # [B,96*96*9]
  xd     = quickgelu(x @ Wd + bd)                 # [B,H,W,96]
  y      = quickgelu(conv3x3(xd, wflat))          # per-sample dynamic grouped conv
  out    = y @ Wu + bu                            # [B,H,W,384]

Sharding: data-parallel over batch B=64 across 8 cores (8 samples/core),
weights replicated.

Key layout/precision choices (v2):
  - x is transposed to channel-major [128, 3, BL*P] on the host and streamed
    as bf16 (halves DMA, removes all on-device transposes).
  - The hypernet is split as wflat = wbase + prompt @ Wh where
    wbase = (emb + bm2) @ Wh + bh is computed exactly on the host.  Only the
    small dynamic part runs on-device, which lets Wh stream in fp8 (e4m3,
    x32 scale) at half the bf16 DMA cost without blowing the error budget.
  - Wh columns are permuted to (tap, o, i) order so each 3x3 tap's weights
    complete early in the stream; the conv for the first 2.5 samples
    "chases" the stream tap by tap (PSUM-bank limited) while the remaining
    halves replay from SBUF afterwards.  Scheduler phase barriers
    (tile_wait_until) keep a chase that waits on the wbase-add DVE chain
    from head-of-line-blocking ready hypernet matmuls.
  - The unadapter output is produced channel-major ([3,128] chunks of C) so
    Wu can be the stationary operand; bu is applied via the PSUM->SBUF copy
    bias; una halves are staggered behind the wave-B convs as PE fillers.
    The host transposes the bf16 result back.
"""
import numpy as np
import ml_dtypes

import concourse.bass as bass
import concourse.tile as tile
import concourse.mybir as mybir
from concourse import bacc
from concourse.bass_utils import run_bass_kernel_spmd

F32 = mybir.dt.float32
BF = mybir.dt.bfloat16
F8 = mybir.dt.float8e4
AF = mybir.ActivationFunctionType
AX = mybir.AxisListType
OP = mybir.AluOpType

# problem constants
B, H, W, C = 64, 28, 28, 384
DIM, E, KK = 96, 64, 3
NCORES = 8
BL = B // NCORES          # samples per core
P = H * W                 # 784 positions per sample
NPOS = BL * P             # 6272 positions per core
NTO = DIM * 9             # 864 (tap, o) pairs
WH_COLS = DIM * NTO       # 82944

WH_FP8 = True             # stream hypernet matrix in fp8 (needs wbase split)
WH_SCALE = 32.0           # Wh pre-scale (keeps fp8 values in normal range)
PV_SCALE = 8.0            # prompt pre-scale
W_SCALE = (WH_SCALE * PV_SCALE) if WH_FP8 else 1.0   # scale carried by w_all

PAIRS_PER_CHUNK = 96      # hypernet (t,o) pairs per Wh DMA chunk (= one tap)
NCHUNK = NTO // PAIRS_PER_CHUNK      # 9
CHUNK_COLS = PAIRS_PER_CHUNK * DIM   # 9216
GRP = 48                  # (t,o) pairs per PSUM group (bank limit)
# conv half-sample tiles [96, 392] that chase the Wh stream (PSUM banks:
# len(CHASE) + pw bufs must be <= 8)
CHASE = [(0, 0), (0, 1), (1, 0), (1, 1), (2, 0)]


def build_nc():
    nc = bacc.Bacc("TRN2", target_bir_lowering=False, debug=False)

    xT_d = nc.dram_tensor("xT", [128, C // 128, NPOS], BF, kind="ExternalInput").ap()
    wm1_d = nc.dram_tensor("wm1", [128, C // 128, E], BF, kind="ExternalInput").ap()
    wd_d = nc.dram_tensor("wd", [128, C // 128, DIM], BF, kind="ExternalInput").ap()
    wm2_d = nc.dram_tensor("wm2", [E, E], F32, kind="ExternalInput").ap()
    wu_d = nc.dram_tensor("wu", [DIM, C], BF, kind="ExternalInput").ap()
    wh_d = nc.dram_tensor("wh", [E, WH_COLS], F8 if WH_FP8 else BF,
                          kind="ExternalInput").ap()
    wbase_d = nc.dram_tensor("wbase", [DIM, NTO, 1], BF, kind="ExternalInput").ap()
    # fused small biases: col 0 = bm1 (rows 0:64), col 1 = bd (rows 0:96),
    # cols 2:5 = bu in three 128-row chunks
    bias_d = nc.dram_tensor("bias", [128, 5], F32, kind="ExternalInput").ap()
    out_d = nc.dram_tensor("out", [128, C // 128, NPOS], BF, kind="ExternalOutput").ap()

    with tile.TileContext(nc) as tc:
        with (
            tc.tile_pool(name="const", bufs=1) as cp,
            tc.tile_pool(name="persist", bufs=1) as pp,
            tc.tile_pool(name="wh", bufs=9) as wh_p,
            tc.tile_pool(name="outp", bufs=3) as out_p,
        ):
            # ---- persistent state ----
            xT_sb = pp.tile([128, C // 128, NPOS], BF)
            xd_pad = pp.tile([DIM, BL, H + 2, W + 2], BF)
            w_all = pp.tile([DIM, NTO, BL], BF)      # [i, (t,o), b] * W_SCALE
            y_sb = pp.tile([DIM, BL, P], BF)
            hsum = pp.tile([E, BL, 2], F32)
            hbar = pp.tile([E, BL], F32)
            hbar_r = pp.tile([E, BL], BF)
            pvec = pp.tile([E, BL], F8 if WH_FP8 else BF)
            h_scr = pp.tile([E, 392], F32)

            # ---- DMA order: h/xd weights, x per sample, rest, Wh chunks ----
            wm1_sb = cp.tile([128, C // 128, E], BF)
            bias_sb = cp.tile([128, 5], F32)
            wd_sb = cp.tile([128, C // 128, DIM], BF)
            wm2_sb = cp.tile([E, E], F32)
            wu_sb = cp.tile([DIM, C], BF)
            wbase_sb = cp.tile([DIM, NTO, 1], BF)

            nc.sync.dma_start(xT_sb[:, :, 0:196], xT_d[:, :, 0:196])
            nc.sync.dma_start(wm1_sb[:], wm1_d)
            nc.sync.dma_start(xT_sb[:, :, 196:392], xT_d[:, :, 196:392])
            nc.sync.dma_start(bias_sb[:], bias_d)
            nc.sync.dma_start(xT_sb[:, :, 392:P], xT_d[:, :, 392:P])
            nc.sync.dma_start(wd_sb[:], wd_d)
            for b in range(1, BL):
                nc.sync.dma_start(xT_sb[:, :, b * P:(b + 1) * P],
                                    xT_d[:, :, b * P:(b + 1) * P])
            nc.sync.dma_start(wm2_sb[:], wm2_d)
            nc.sync.dma_start(wbase_sb[:], wbase_d)
            nc.sync.dma_start(wu_sb[:], wu_d)

            bm1 = bias_sb[0:E, 0:1]
            bd = bias_sb[0:DIM, 1:2]

            # warm-up input first: its memset unblocks the PE ramp matmuls
            warm = cp.tile([E, 392], BF)
            nc.vector.memset(warm[:], 0.0)
            # zero the conv halo borders of xd_pad (interior is overwritten)
            nc.vector.memset(xd_pad[:, :, 0:30:29, :], 0.0)
            nc.vector.memset(xd_pad[:, :, 1:29, 0:30:29], 0.0)
            zeros = cp.tile([E, 1], F32)
            nc.vector.memset(zeros[:], 0.0)
            h_scr2 = pp.tile([E, 392], F32)

            # ---- per sample: meta-net h sums + xd = quickgelu(x@Wd+bd).
            # ppm (the prompt matmul) is issued before the last xd half so
            # pvec is ready the moment the h sums complete. ----
            hp_ctx = tc.tile_pool(name="ps_h", bufs=3, space="PSUM")
            ps_h = hp_ctx.__enter__()
            xp_ctx = tc.tile_pool(name="ps_x", bufs=3, space="PSUM")
            ps_x = xp_ctx.__enter__()

            def make_h(b, h2):
                lo = b * P + h2 * 392
                ph = ps_h.tile([E, 392], F32, name="ph", tag="ph")
                for c in range(C // 128):
                    nc.tensor.matmul(ph[:], wm1_sb[:, c, :], xT_sb[:, c, lo:lo + 392],
                                     start=(c == 0), stop=(c == 2))
                # relu+bias+spatial-sum on DVE (ACT is busy with gelus,
                # and pvec is on the critical path)
                nc.vector.scalar_tensor_tensor(
                    h_scr2[:], ph[:], bm1, zeros[:].to_broadcast((E, 392)),
                    op0=OP.add, op1=OP.max,
                    accum_out=hsum[:, b, h2:h2 + 1])

            def make_xd(b, h2):
                lo = b * P + h2 * 392
                px = ps_x.tile([DIM, 392], F32, name="px", tag="px")
                for c in range(C // 128):
                    nc.tensor.matmul(px[:], wd_sb[:, c, :], xT_sb[:, c, lo:lo + 392],
                                     start=(c == 0), stop=(c == 2))
                nc.scalar.activation(
                    xd_pad[:, b, 1 + h2 * 14: 15 + h2 * 14, 1:29],
                    px[:].rearrange("p (r c) -> p r c", r=14),
                    AF.Gelu_apprx_sigmoid, bias=bd)

            # dep-free warmup matmuls on the ph ring: the PE p-state needs
            # ~3us of gapless work to reach full speed, so burn the dead time
            # before the first x tile lands ramping up (ring of 3 buffers ->
            # no WAW semaphore gaps between warmups)
            with tc.high_priority():
                for k in range(14):
                    phw = ps_h.tile([E, 392], F32, name=f"warm{k}", tag="ph")
                    nc.tensor.matmul(phw[:], warm[:, 0:E], warm[:],
                                     start=True, stop=True)

            # first h half runs as two 196-column pieces so the PE starts as
            # soon as the first quarter of sample 0 lands
            with tc.high_priority():
                ph0 = ps_h.tile([E, 392], F32, name="ph", tag="ph")
                for piece in range(2):
                    for c in range(C // 128):
                        nc.tensor.matmul(
                            ph0[:, piece * 196:(piece + 1) * 196],
                            wm1_sb[:, c, :],
                            xT_sb[:, c, piece * 196:(piece + 1) * 196],
                            start=(c == 0), stop=(c == 2))
                nc.vector.scalar_tensor_tensor(
                    h_scr2[:], ph0[:], bm1, zeros[:].to_broadcast((E, 392)),
                    op0=OP.add, op1=OP.max, accum_out=hsum[:, 0, 0:1])
            make_xd(0, 0)
            for b in range(BL):
                if b < BL - 1:
                    for h2 in range(2):
                        if b == 0 and h2 == 0:
                            continue
                        with tc.high_priority():
                            make_h(b, h2)
                        make_xd(b, h2)
                else:
                    with tc.high_priority():
                        make_h(b, 0)
                        make_h(b, 1)

            # ---- prompt -> pvec (scaled; no emb/bias: wbase covers them).
            # The prompt matmul contracts the two spatial-half sums directly
            # (wm2 is uploaded fp32, pre-scaled by 1/P) — no reduce/rescale
            # sits on the pvec critical path. ----
            make_xd(BL - 1, 0)
            with tc.high_priority():
                ppm = ps_h.tile([E, BL], F32, name="ppm", tag="ppm", bufs=1)
                for j in range(2):
                    nc.tensor.matmul(ppm[:], wm2_sb[:], hsum[:, :, j],
                                     start=(j == 0), stop=(j == 1))
                nc.scalar.activation(pvec[:], ppm[:], AF.Copy, scale=PV_SCALE)
            make_xd(BL - 1, 1)
            xp_ctx.__exit__(None, None, None)
            hp_ctx.__exit__(None, None, None)

            # ---- Wh stream: hypernet + wbase add + wave-A conv chase ----
            pca_ctx = tc.tile_pool(name="ps_ca", bufs=1, space="PSUM")
            ps_ca = pca_ctx.__enter__()
            pca = [ps_ca.tile([DIM, 392], F32, name=f"pca{k}") for k in range(len(CHASE))]
            pw_ctx = tc.tile_pool(name="ps_w", bufs=3, space="PSUM")
            ps_w = pw_ctx.__enter__()

            def conv_half(pc, s, h2, t):
                dy, dx = t // 3, t % 3
                nc.tensor.matmul(
                    pc,
                    w_all[:, t * DIM:(t + 1) * DIM, s],
                    xd_pad[:, s, h2 * 14 + dy: h2 * 14 + dy + 14, dx:dx + 28],
                    start=(t == 0), stop=(t == 8))

            def chase(t):
                for k, (s, h2) in enumerate(CHASE):
                    conv_half(pca[k], s, h2, t)

            for jc in range(NCHUNK):
              # logical-phase barrier: force the scheduler to emit each
              # iteration's PE work in source order so a chase waiting on its
              # wbase-add never head-of-line-blocks ready hypernet matmuls
              with tc.tile_wait_until(ms=jc + 1):
                whc = wh_p.tile([E, CHUNK_COLS], F8 if WH_FP8 else BF, tag="whc")
                nc.sync.dma_start(whc[:], wh_d[:, jc * CHUNK_COLS:(jc + 1) * CHUNK_COLS])
                for half in range(PAIRS_PER_CHUNK // GRP):
                    pw = ps_w.tile([DIM, GRP * BL], F32, name="pw", tag="pw")
                    for g in range(GRP):
                        gg = half * GRP + g
                        nc.tensor.matmul(pw[:, g * BL:(g + 1) * BL],
                                         whc[:, gg * DIM:(gg + 1) * DIM], pvec[:],
                                         start=True, stop=True)
                    g0 = jc * PAIRS_PER_CHUNK + half * GRP
                    nc.vector.scalar_tensor_tensor(
                        w_all[:, g0:g0 + GRP, :],
                        pw[:].rearrange("p (g b) -> p g b", g=GRP),
                        1.0,
                        wbase_sb[:, g0:g0 + GRP, :].to_broadcast((DIM, GRP, BL)),
                        op0=OP.mult, op1=OP.add)
                # conv chase, lagging two taps so it never waits on the
                # wbase-add DVE chain of the chunk just streamed
                if jc > 1:
                    chase(jc - 2)
            with tc.tile_wait_until(ms=NCHUNK + 1):
                chase(NCHUNK - 2)
                chase(NCHUNK - 1)
            pw_ctx.__exit__(None, None, None)

            # ---- finish: gelu + unadapter + out at half-sample granularity.
            # una halves are staggered behind the conv so they fill PE stalls
            # while ACT runs the gelus. ----
            po_ctx = tc.tile_pool(name="ps_o", bufs=3, space="PSUM")
            ps_o = po_ctx.__enter__()
            neng = 0
            ob_tiles = {}

            def gelu_half(pc, s, h2):
                nc.scalar.activation(y_sb[:, s, h2 * 392:(h2 + 1) * 392],
                                     pc, AF.Gelu_apprx_sigmoid,
                                     scale=1.0 / W_SCALE)

            def una_half(s, h2):
                nonlocal neng
                if s not in ob_tiles:
                    ob_tiles[s] = out_p.tile([128, C // 128, P], BF, tag="ob",
                                             name=f"ob{s}")
                ob = ob_tiles[s]
                for q in range(C // 128):
                    po = ps_o.tile([128, 392], F32, name="po", tag="po")
                    nc.tensor.matmul(po[:], wu_sb[:, q * 128:(q + 1) * 128],
                                     y_sb[:, s, h2 * 392:(h2 + 1) * 392],
                                     start=True, stop=True)
                    dst = ob[:, q, h2 * 392:(h2 + 1) * 392]
                    if neng % 2 == 0:
                        nc.scalar.activation(dst, po[:], AF.Identity,
                                             bias=bias_sb[:, 2 + q:3 + q])
                    else:
                        nc.vector.scalar_tensor_tensor(
                            dst, po[:], 1.0,
                            bias_sb[:, 2 + q:3 + q].to_broadcast((128, 392)),
                            op0=OP.mult, op1=OP.add)
                    neng += 1
                    if h2 == 1:
                        nc.sync.dma_start(out_d[:, q, s * P:(s + 1) * P],
                                          ob[:, q, :])

            for k, (s, h2) in enumerate(CHASE):
                gelu_half(pca[k], s, h2)
            pend = list(CHASE)
            wave_b = [sh for s in range(BL) for sh in ((s, 0), (s, 1))
                      if sh not in CHASE]
            for k, (s, h2) in enumerate(wave_b):
                pc = pca[k % len(pca)]
                for t in range(9):
                    conv_half(pc, s, h2, t)
                gelu_half(pc, s, h2)
                for _ in range(2 if len(pend) > 1 else 1):
                    if pend:
                        una_half(*pend.pop(0))
                pend.append((s, h2))
            for s, h2 in pend:
                una_half(s, h2)
            po_ctx.__exit__(None, None, None)
            pca_ctx.__exit__(None, None, None)

    nc.compile()
    return nc


_NC_CACHE = None


def _get_nc():
    global _NC_CACHE
    if _NC_CACHE is None:
        _NC_CACHE = build_nc()
    return _NC_CACHE


def _prep_inputs(x, Wd, bd, Wm1, bm1, Wm2, bm2, Wh, bh, emb, Wu, bu):
    """Host-side prep: permute weights, precompute wbase, shard x."""
    f32 = np.float32
    bf16 = ml_dtypes.bfloat16

    # Wh columns: original order (o, i, t) -> (t, o, i)
    whp = np.asarray(Wh, f32).reshape(E, DIM, DIM, 9)          # (e, o, i, t)
    whp = whp.transpose(0, 3, 1, 2).reshape(E, WH_COLS)        # (e, (t, o, i))
    # wbase = (emb + bm2) @ Wh + bh, in [i, (t, o)] layout, carrying W_SCALE
    wb = (np.asarray(emb, f32) + np.asarray(bm2, f32)) @ np.asarray(Wh, f32) \
        + np.asarray(bh, f32)                                  # [(o, i, t)]
    wb = wb.reshape(DIM, DIM, 9).transpose(1, 2, 0)            # (i, t, o)
    wb = np.ascontiguousarray(wb.reshape(DIM, NTO, 1) * W_SCALE)

    if WH_FP8:
        wh_up = (whp * WH_SCALE).astype(ml_dtypes.float8_e4m3)
    else:
        wh_up = whp.astype(bf16)

    bias = np.zeros((128, 5), f32)
    bias[:E, 0] = np.asarray(bm1, f32)
    bias[:DIM, 1] = np.asarray(bd, f32)
    bias[:, 2:5] = np.asarray(bu, f32).reshape(C // 128, 128).T

    def cmajor(a):  # [C, k] -> [128, 3, k]
        return np.ascontiguousarray(
            np.asarray(a, f32).reshape(C // 128, 128, -1).transpose(1, 0, 2)
        ).astype(bf16)

    shared = {
        "wm1": cmajor(Wm1),
        "wd": cmajor(Wd),
        "wm2": np.ascontiguousarray(np.asarray(Wm2, f32) / P),
        "wu": np.asarray(Wu, f32).astype(bf16),
        "wh": wh_up,
        "wbase": wb.astype(bf16),
        "bias": bias,
    }
    xs = np.asarray(x, f32).reshape(B, P, C)
    in_maps = []
    for k in range(NCORES):
        m = dict(shared)
        xc = xs[k * BL:(k + 1) * BL].reshape(NPOS, C)          # [n, c]
        xc = xc.T.reshape(C // 128, 128, NPOS).transpose(1, 0, 2)
        m["xT"] = np.ascontiguousarray(xc).astype(bf16)
        in_maps.append(m)
    return in_maps


def _run(inputs, **spmd_kwargs):
    nc = _get_nc()
    in_maps = _prep_inputs(**inputs)
    res = run_bass_kernel_spmd(nc, in_maps, core_ids=list(range(NCORES)), **spmd_kwargs)
    outs = []
    for r in res.results:
        o = np.asarray(r["out"], dtype=np.float32)             # [128, 3, NPOS]
        o = o.transpose(1, 0, 2).reshape(C, NPOS).T            # [NPOS, C]
        outs.append(o)
    out = np.concatenate(outs, 0)
    return out.reshape(B, H, W, C), res


def kernel(**inputs) -> np.ndarray:
    out, _ = _run(inputs)
    return out


# revision 57
# speedup vs baseline: 1.0339x; 1.0056x over previous
"""Trainium2 Bass kernel for Convpass-swin hypernet fused adapter.

Reference computation (per batch sample):
  h      = relu(x @ Wm1 + bm1)                    # [B,H,W,64]
  prompt = mean_hw(h) @ Wm2 + bm2                 # [B,64]  (mean commutes with matmul)
  wflat  = (emb + prompt) @ Wh + bh               # [B,96*96*9]
  xd     = quickgelu(x @ Wd + bd)                 # [B,H,W,96]
  y      = quickgelu(conv3x3(xd, wflat))          # per-sample dynamic grouped conv
  out    = y @ Wu + bu                            # [B,H,W,384]

Sharding: data-parallel over batch B=64 across 8 cores (8 samples/core),
weights replicated.

Key layout/precision choices (v2):
  - x is transposed to channel-major [128, 3, BL*P] on the host and streamed
    as bf16 (halves DMA, removes all on-device transposes).
  - The hypernet is split as wflat = wbase + prompt @ Wh where
    wbase = (emb + bm2) @ Wh + bh is computed exactly on the host.  Only the
    small dynamic part runs on-device, which lets Wh stream in fp8 (e4m3,
    x32 scale) at half the bf16 DMA cost without blowing the error budget.
  - Wh columns are permuted to (tap, o, i) order so each 3x3 tap's weights
    complete early in the stream; the conv for the first 2.5 samples
    "chases" the stream tap by tap (PSUM-bank limited) while the remaining
    halves replay from SBUF afterwards.  Scheduler phase barriers
    (tile_wait_until) keep a chase that waits on the wbase-add DVE chain
    from head-of-line-blocking ready hypernet matmuls.
  - The unadapter output is produced channel-major ([3,128] chunks of C) so
    Wu can be the stationary operand; bu is applied via the PSUM->SBUF copy
    bias; una halves are staggered behind the wave-B convs as PE fillers.
    The host transposes the bf16 result back.
"""
import numpy as np
import ml_dtypes

import concourse.bass as bass
import concourse.tile as tile
import concourse.mybir as mybir
from concourse import bacc
from concourse.bass_utils import run_bass_kernel_spmd

F32 = mybir.dt.float32
BF = mybir.dt.bfloat16
F8 = mybir.dt.float8e4
AF = mybir.ActivationFunctionType
AX = mybir.AxisListType
OP = mybir.AluOpType

# problem constants
B, H, W, C = 64, 28, 28, 384
DIM, E, KK = 96, 64, 3
NCORES = 8
BL = B // NCORES          # samples per core
P = H * W                 # 784 positions per sample
NPOS = BL * P             # 6272 positions per core
NTO = DIM * 9             # 864 (tap, o) pairs
WH_COLS = DIM * NTO       # 82944

WH_FP8 = True             # stream hypernet matrix in fp8 (needs wbase split)
WH_SCALE = 32.0           # Wh pre-scale (keeps fp8 values in normal range)
PV_SCALE = 8.0            # prompt pre-scale
W_SCALE = (WH_SCALE * PV_SCALE) if WH_FP8 else 1.0   # scale carried by w_all

PAIRS_PER_CHUNK = 96      # hypernet (t,o) pairs per Wh DMA chunk (= one tap)
NCHUNK = NTO // PAIRS_PER_CHUNK      # 9
CHUNK_COLS = PAIRS_PER_CHUNK * DIM   # 9216
GRP = 48                  # (t,o) pairs per PSUM group (bank limit)
# conv half-sample tiles [96, 392] that chase the Wh stream (PSUM banks:
# len(CHASE) + pw bufs must be <= 8)
CHASE = [(0, 0), (0, 1), (1, 0), (1, 1), (2, 0)]


def build_nc():
    nc = bacc.Bacc("TRN2", target_bir_lowering=False, debug=False)

    xT_d = nc.dram_tensor("xT", [128, C // 128, NPOS], BF, kind="ExternalInput").ap()
    wm1_d = nc.dram_tensor("wm1", [128, C // 128, E], BF, kind="ExternalInput").ap()
    wd_d = nc.dram_tensor("wd", [128, C // 128, DIM], BF, kind="ExternalInput").ap()
    wm2_d = nc.dram_tensor("wm2", [E, E], F32, kind="ExternalInput").ap()
    wu_d = nc.dram_tensor("wu", [DIM, C], BF, kind="ExternalInput").ap()
    wh_d = nc.dram_tensor("wh", [E, WH_COLS], F8 if WH_FP8 else BF,
                          kind="ExternalInput").ap()
    wbase_d = nc.dram_tensor("wbase", [DIM, NTO, 1], BF, kind="ExternalInput").ap()
    # fused small biases: col 0 = bm1 (rows 0:64), col 1 = bd (rows 0:96),
    # cols 2:5 = bu in three 128-row chunks
    bias_d = nc.dram_tensor("bias", [128, 5], F32, kind="ExternalInput").ap()
    out_d = nc.dram_tensor("out", [128, C // 128, NPOS], BF, kind="ExternalOutput").ap()

    with tile.TileContext(nc) as tc:
        with (
            tc.tile_pool(name="const", bufs=1) as cp,
            tc.tile_pool(name="persist", bufs=1) as pp,
            tc.tile_pool(name="wh", bufs=9) as wh_p,
            tc.tile_pool(name="outp", bufs=3) as out_p,
        ):
            # ---- persistent state ----
            xT_sb = pp.tile([128, C // 128, NPOS], BF)
            xd_pad = pp.tile([DIM, BL, H + 2, W + 2], BF)
            w_all = pp.tile([DIM, NTO, BL], BF)      # [i, (t,o), b] * W_SCALE
            y_sb = pp.tile([DIM, BL, P], BF)
            hsum = pp.tile([E, BL, 2], F32)
            hbar = pp.tile([E, BL], F32)
            hbar_r = pp.tile([E, BL], BF)
            pvec = pp.tile([E, BL], F8 if WH_FP8 else BF)
            h_scr = pp.tile([E, 392], F32)

            # ---- DMA order: h/xd weights, x per sample, rest, Wh chunks ----
            wm1_sb = cp.tile([128, C // 128, E], BF)
            bias_sb = cp.tile([128, 5], F32)
            wd_sb = cp.tile([128, C // 128, DIM], BF)
            wm2_sb = cp.tile([E, E], F32)
            wu_sb = cp.tile([DIM, C], BF)
            wbase_sb = cp.tile([DIM, NTO, 1], BF)

            nc.sync.dma_start(xT_sb[:, :, 0:196], xT_d[:, :, 0:196])
            nc.sync.dma_start(wm1_sb[:], wm1_d)
            nc.sync.dma_start(xT_sb[:, :, 196:392], xT_d[:, :, 196:392])
            nc.sync.dma_start(bias_sb[:], bias_d)
            nc.sync.dma_start(xT_sb[:, :, 392:P], xT_d[:, :, 392:P])
            nc.sync.dma_start(wd_sb[:], wd_d)
            for b in range(1, BL):
                nc.sync.dma_start(xT_sb[:, :, b * P:(b + 1) * P],
                                    xT_d[:, :, b * P:(b + 1) * P])
            nc.sync.dma_start(wm2_sb[:], wm2_d)
            nc.sync.dma_start(wbase_sb[:], wbase_d)
            nc.sync.dma_start(wu_sb[:], wu_d)

            bm1 = bias_sb[0:E, 0:1]
            bd = bias_sb[0:DIM, 1:2]

            # warm-up input first: its memset unblocks the PE ramp matmuls
            warm = cp.tile([E, 392], BF)
            nc.vector.memset(warm[:], 0.0)
            # zero the conv halo borders of xd_pad (interior is overwritten)
            nc.vector.memset(xd_pad[:, :, 0:30:29, :], 0.0)
            nc.vector.memset(xd_pad[:, :, 1:29, 0:30:29], 0.0)
            zeros = cp.tile([E, 1], F32)
            nc.vector.memset(zeros[:], 0.0)
            h_scr2 = pp.tile([E, 392], F32)

            # ---- per sample: meta-net h sums + xd = quickgelu(x@Wd+bd).
            # ppm (the prompt matmul) is issued before the last xd half so
            # pvec is ready the moment the h sums complete. ----
            hp_ctx = tc.tile_pool(name="ps_h", bufs=3, space="PSUM")
            ps_h = hp_ctx.__enter__()
            xp_ctx = tc.tile_pool(name="ps_x", bufs=3, space="PSUM")
            ps_x = xp_ctx.__enter__()

            def make_h(b, h2):
                lo = b * P + h2 * 392
                ph = ps_h.tile([E, 392], F32, name="ph", tag="ph")
                for c in range(C // 128):
                    nc.tensor.matmul(ph[:], wm1_sb[:, c, :], xT_sb[:, c, lo:lo + 392],
                                     start=(c == 0), stop=(c == 2))
                # relu+bias+spatial-sum on DVE (ACT is busy with gelus,
                # and pvec is on the critical path)
                nc.vector.scalar_tensor_tensor(
                    h_scr2[:], ph[:], bm1, zeros[:].to_broadcast((E, 392)),
                    op0=OP.add, op1=OP.max,
                    accum_out=hsum[:, b, h2:h2 + 1])

            def make_xd(b, h2):
                lo = b * P + h2 * 392
                px = ps_x.tile([DIM, 392], F32, name="px", tag="px")
                for c in range(C // 128):
                    nc.tensor.matmul(px[:], wd_sb[:, c, :], xT_sb[:, c, lo:lo + 392],
                                     start=(c == 0), stop=(c == 2))
                nc.scalar.activation(
                    xd_pad[:, b, 1 + h2 * 14: 15 + h2 * 14, 1:29],
                    px[:].rearrange("p (r c) -> p r c", r=14),
                    AF.Gelu_apprx_sigmoid, bias=bd)

            # dep-free warmup matmuls on the ph ring: the PE p-state needs
            # ~3us of gapless work to reach full speed, so burn the dead time
            # before the first x tile lands ramping up (ring of 3 buffers ->
            # no WAW semaphore gaps between warmups)
            with tc.high_priority():
                for k in range(14):
                    phw = ps_h.tile([E, 392], F32, name=f"warm{k}", tag="ph")
                    nc.tensor.matmul(phw[:], warm[:, 0:E], warm[:],
                                     start=True, stop=True)

            # first h half runs as two 196-column pieces so the PE starts as
            # soon as the first quarter of sample 0 lands
            with tc.high_priority():
                ph0 = ps_h.tile([E, 392], F32, name="ph", tag="ph")
                for piece in range(2):
                    for c in range(C // 128):
                        nc.tensor.matmul(
                            ph0[:, piece * 196:(piece + 1) * 196],
                            wm1_sb[:, c, :],
                            xT_sb[:, c, piece * 196:(piece + 1) * 196],
                            start=(c == 0), stop=(c == 2))
                nc.vector.scalar_tensor_tensor(
                    h_scr2[:], ph0[:], bm1, zeros[:].to_broadcast((E, 392)),
                    op0=OP.add, op1=OP.max, accum_out=hsum[:, 0, 0:1])
            make_xd(0, 0)
            for b in range(BL):
                if b < BL - 1:
                    for h2 in range(2):
                        if b == 0 and h2 == 0:
                            continue
                        with tc.high_priority():
                            make_h(b, h2)
                        make_xd(b, h2)
                else:
                    with tc.high_priority():
                        make_h(b, 0)
                        make_h(b, 1)

            # ---- prompt -> pvec (scaled; no emb/bias: wbase covers them).
            # The prompt matmul contracts the two spatial-half sums directly
            # (wm2 is uploaded fp32, pre-scaled by 1/P) — no reduce/rescale
            # sits on the pvec critical path. ----
            make_xd(BL - 1, 0)
            with tc.high_priority():
                ppm = ps_h.tile([E, BL], F32, name="ppm", tag="ppm", bufs=1)
                for j in range(2):
                    nc.tensor.matmul(ppm[:], wm2_sb[:], hsum[:, :, j],
                                     start=(j == 0), stop=(j == 1))
                nc.scalar.activation(pvec[:], ppm[:], AF.Copy, scale=PV_SCALE)
            make_xd(BL - 1, 1)
            xp_ctx.__exit__(None, None, None)
            hp_ctx.__exit__(None, None, None)

            # ---- Wh stream: hypernet + wbase add + wave-A conv chase ----
            pca_ctx = tc.tile_pool(name="ps_ca", bufs=1, space="PSUM")
            ps_ca = pca_ctx.__enter__()
            pca = [ps_ca.tile([DIM, 392], F32, name=f"pca{k}") for k in range(len(CHASE))]
            pw_ctx = tc.tile_pool(name="ps_w", bufs=3, space="PSUM")
            ps_w = pw_ctx.__enter__()

            def conv_half(pc, s, h2, t):
                dy, dx = t // 3, t % 3
                nc.tensor.matmul(
                    pc,
                    w_all[:, t * DIM:(t + 1) * DIM, s],
                    xd_pad[:, s, h2 * 14 + dy: h2 * 14 + dy + 14, dx:dx + 28],
                    start=(t == 0), stop=(t == 8))

            def chase(t):
                for k, (s, h2) in enumerate(CHASE):
                    conv_half(pca[k], s, h2, t)

            for jc in range(NCHUNK):
              # logical-phase barrier: force the scheduler to emit each
              # iteration's PE work in source order so a chase waiting on its
              # wbase-add never head-of-line-blocks ready hypernet matmuls
              with tc.tile_wait_until(ms=jc + 1):
                whc = wh_p.tile([E, CHUNK_COLS], F8 if WH_FP8 else BF, tag="whc")
                nc.sync.dma_start(whc[:], wh_d[:, jc * CHUNK_COLS:(jc + 1) * CHUNK_COLS])
                for half in range(PAIRS_PER_CHUNK // GRP):
                    pw = ps_w.tile([DIM, GRP * BL], F32, name="pw", tag="pw")
                    for g in range(GRP):
                        gg = half * GRP + g
                        nc.tensor.matmul(pw[:, g * BL:(g + 1) * BL],
                                         whc[:, gg * DIM:(gg + 1) * DIM], pvec[:],
                                         start=True, stop=True)
                    g0 = jc * PAIRS_PER_CHUNK + half * GRP
                    nc.vector.scalar_tensor_tensor(
                        w_all[:, g0:g0 + GRP, :],
                        pw[:].rearrange("p (g b) -> p g b", g=GRP),
                        1.0,
                        wbase_sb[:, g0:g0 + GRP, :].to_broadcast((DIM, GRP, BL)),
                        op0=OP.mult, op1=OP.add)
                # conv chase, lagging two taps so it never waits on the
                # wbase-add DVE chain of the chunk just streamed
                if jc > 1:
                    chase(jc - 2)
            with tc.tile_wait_until(ms=NCHUNK + 1):
                chase(NCHUNK - 2)
                chase(NCHUNK - 1)

            # ---- finish: gelu + unadapter + out at half-sample granularity.
            # una halves are staggered behind the conv so they fill PE stalls
            # while ACT runs the gelus. ----
            neng = 0
            ob_tiles = {}

            def gelu_half(pc, s, h2):
                nc.scalar.activation(y_sb[:, s, h2 * 392:(h2 + 1) * 392],
                                     pc, AF.Gelu_apprx_sigmoid,
                                     scale=1.0 / W_SCALE)

            def una_half(s, h2):
                nonlocal neng
                if s not in ob_tiles:
                    ob_tiles[s] = out_p.tile([128, C // 128, P], BF, tag="ob",
                                             name=f"ob{s}")
                ob = ob_tiles[s]
                for q in range(C // 128):
                    po = ps_w.tile([128, 392], F32, name="po", tag="pw")
                    nc.tensor.matmul(po[:], wu_sb[:, q * 128:(q + 1) * 128],
                                     y_sb[:, s, h2 * 392:(h2 + 1) * 392],
                                     start=True, stop=True)
                    dst = ob[:, q, h2 * 392:(h2 + 1) * 392]
                    if neng % 2 == 0:
                        nc.scalar.activation(dst, po[:], AF.Identity,
                                             bias=bias_sb[:, 2 + q:3 + q])
                    else:
                        nc.vector.scalar_tensor_tensor(
                            dst, po[:], 1.0,
                            bias_sb[:, 2 + q:3 + q].to_broadcast((128, 392)),
                            op0=OP.mult, op1=OP.add)
                    neng += 1
                    if h2 == 1:
                        nc.sync.dma_start(out_d[:, q, s * P:(s + 1) * P],
                                          ob[:, q, :])

            for k, (s, h2) in enumerate(CHASE):
                gelu_half(pca[k], s, h2)
            pend = list(CHASE)
            wave_b = [sh for s in range(BL) for sh in ((s, 0), (s, 1))
                      if sh not in CHASE]
            for k, (s, h2) in enumerate(wave_b):
                if k < 3:
                    # the pw banks free up right after the last wbase-add;
                    # start the first replays there instead of waiting for
                    # the chase gelus to drain the pca ring
                    pc = ps_w.tile([DIM, 392], F32, name=f"pcw{k}",
                                   tag="pw")[:]
                else:
                    pc = pca[(k - 3) % len(pca)]
                for t in range(9):
                    conv_half(pc, s, h2, t)
                gelu_half(pc, s, h2)
                for _ in range(2 if len(pend) > 1 else 1):
                    if pend:
                        una_half(*pend.pop(0))
                pend.append((s, h2))
            for s, h2 in pend:
                una_half(s, h2)
            pw_ctx.__exit__(None, None, None)
            pca_ctx.__exit__(None, None, None)

    nc.compile()
    return nc


_NC_CACHE = None


def _get_nc():
    global _NC_CACHE
    if _NC_CACHE is None:
        _NC_CACHE = build_nc()
    return _NC_CACHE


def _prep_inputs(x, Wd, bd, Wm1, bm1, Wm2, bm2, Wh, bh, emb, Wu, bu):
    """Host-side prep: permute weights, precompute wbase, shard x."""
    f32 = np.float32
    bf16 = ml_dtypes.bfloat16

    # Wh columns: original order (o, i, t) -> (t, o, i)
    whp = np.asarray(Wh, f32).reshape(E, DIM, DIM, 9)          # (e, o, i, t)
    whp = whp.transpose(0, 3, 1, 2).reshape(E, WH_COLS)        # (e, (t, o, i))
    # wbase = (emb + bm2) @ Wh + bh, in [i, (t, o)] layout, carrying W_SCALE
    wb = (np.asarray(emb, f32) + np.asarray(bm2, f32)) @ np.asarray(Wh, f32) \
        + np.asarray(bh, f32)                                  # [(o, i, t)]
    wb = wb.reshape(DIM, DIM, 9).transpose(1, 2, 0)            # (i, t, o)
    wb = np.ascontiguousarray(wb.reshape(DIM, NTO, 1) * W_SCALE)

    if WH_FP8:
        wh_up = (whp * WH_SCALE).astype(ml_dtypes.float8_e4m3)
    else:
        wh_up = whp.astype(bf16)

    bias = np.zeros((128, 5), f32)
    bias[:E, 0] = np.asarray(bm1, f32)
    bias[:DIM, 1] = np.asarray(bd, f32)
    bias[:, 2:5] = np.asarray(bu, f32).reshape(C // 128, 128).T

    def cmajor(a):  # [C, k] -> [128, 3, k]
        return np.ascontiguousarray(
            np.asarray(a, f32).reshape(C // 128, 128, -1).transpose(1, 0, 2)
        ).astype(bf16)

    shared = {
        "wm1": cmajor(Wm1),
        "wd": cmajor(Wd),
        "wm2": np.ascontiguousarray(np.asarray(Wm2, f32) / P),
        "wu": np.asarray(Wu, f32).astype(bf16),
        "wh": wh_up,
        "wbase": wb.astype(bf16),
        "bias": bias,
    }
    xs = np.asarray(x, f32).reshape(B, P, C)
    in_maps = []
    for k in range(NCORES):
        m = dict(shared)
        xc = xs[k * BL:(k + 1) * BL].reshape(NPOS, C)          # [n, c]
        xc = xc.T.reshape(C // 128, 128, NPOS).transpose(1, 0, 2)
        m["xT"] = np.ascontiguousarray(xc).astype(bf16)
        in_maps.append(m)
    return in_maps


def _run(inputs, **spmd_kwargs):
    nc = _get_nc()
    in_maps = _prep_inputs(**inputs)
    res = run_bass_kernel_spmd(nc, in_maps, core_ids=list(range(NCORES)), **spmd_kwargs)
    outs = []
    for r in res.results:
        o = np.asarray(r["out"], dtype=np.float32)             # [128, 3, NPOS]
        o = o.transpose(1, 0, 2).reshape(C, NPOS).T            # [NPOS, C]
        outs.append(o)
    out = np.concatenate(outs, 0)
    return out.reshape(B, H, W, C), res


def kernel(**inputs) -> np.ndarray:
    out, _ = _run(inputs)
    return out
